# revision 12
# baseline (speedup 1.0000x reference)
"""Trainium2 Bass kernel for linear attention over external memory.

Computes out = x @ (keys^T @ vals) for
  x [4, 2048, 1024] f32, keys/vals [65536, 1024] f32.

Sharding across 8 NeuronCores: keys/vals sharded along the memory dim M
(8192 rows per core); each core computes a partial kv = keys_s^T @ vals_s,
AllReduces kv in bf16, then computes its token shard of x @ kv
(x sharded by token, 1024 rows per core).

All inputs are cast to bf16 on the host (halves DMA traffic, enables FWL
fast weight loads). x is transposed on the host. k/v chunks are packed
on the host into groups (v: 4 chunks, k: 2 chunks) so each input DMA
moves a large contiguous block — dma_start costs ~0.7us of sequencer
issue time each — and k (sync) / v+xT (scalar) streams issue on separate
queues. The first few groups are split into per-chunk DMAs so the PE can
start as soon as a small transfer lands.

Stage 2 accumulates kv fully in PSUM over all 64 k-chunks in 4 phases of
3/2/2/1 kv row-blocks (phase 0 is larger so its compute covers the input
DMA ramp; the last phase is small so the final AllReduce is small).
Phase results are cast to bf16 (vector+scalar halves in parallel) and
AllReduced while later phases compute; all collective-path ops sit on
the gpsimd queue in program order.

Stage 4 (out = x @ kv) runs j-outer with the last row-block deferred:
j0-6 partials for both column halves are copied into the output staging
tiles (freeing all PSUM banks) while the last AllReduce is in flight;
once it lands, 16 independent single-matmul j7 groups stream through the
PE back-to-back and vector/gpsimd add them into the staging tiles.
"""

import numpy as np

# Problem shapes (hardcoded per contract).
B, S, D = 4, 2048, 1024
M = 65536
NCORES = 8
P = 128
T = (B * S) // NCORES          # 1024 tokens per core
KM = M // NCORES               # 8192 memory rows per core
NCH = KM // P                  # 64 k-chunks
DB = D // P                    # 8 d / kv-row blocks
TCH = T // P                   # 8 token chunks
HALF = D // 2                  # 512
VG = 4                         # chunks per v DMA group
KG = 2                         # chunks per k DMA group
NVG = NCH // VG                # 16 v groups
NKG = NCH // KG                # 32 k groups

# Stage-2 phases: number of kv row-blocks finished per phase.
PH_BLOCKS = [3, 2, 2, 1]
PH_J0 = [0, 3, 5, 7]           # first row-block of each phase

_CACHE = {}


def _build_nc():
    import concourse.bacc as bacc
    import concourse.tile as tile
    from concourse import mybir

    f32 = mybir.dt.float32
    bf16 = mybir.dt.bfloat16
    ACT_COPY = mybir.ActivationFunctionType.Copy

    nc = bacc.Bacc("TRN2", target_bir_lowering=False, debug=False,
                   num_devices=NCORES)

    # k per phase, v, and xT arrive pre-grouped from the host.
    ks_d = [nc.dram_tensor(f"ks{p}", [NKG, P, KG * PH_BLOCKS[p] * P], bf16,
                           kind="ExternalInput")
            for p in range(4)]
    vs_d = nc.dram_tensor("vs", [NVG, P, VG * D], bf16, kind="ExternalInput")
    xT_d = nc.dram_tensor("xT", [DB, P, T], bf16, kind="ExternalInput")
    out_d = nc.dram_tensor("out", [T, D], f32, kind="ExternalOutput")

    with tile.TileContext(nc) as tc:
        with (
            tc.tile_pool(name="const", bufs=1) as const,
            tc.tile_pool(name="vpool", bufs=NVG) as vpool,
            tc.tile_pool(name="kpool", bufs=2) as kpool,
            tc.tile_pool(name="xpool", bufs=DB) as xpool,
            tc.tile_pool(name="kvdp", bufs=2) as kvdp,
            tc.tile_pool(name="kvrp", bufs=DB) as kvrp,
            tc.tile_pool(name="outp", bufs=TCH) as outp,
            tc.tile_pool(name="ps", bufs=8, space="PSUM") as ps,
            tc.tile_pool(name="dram", bufs=10, space="DRAM") as dram,
        ):
            # Warm-up collective: arms the ncfw collective stream so the
            # first real AllReduce trigger doesn't pay the wake-up.
            warm = const.tile([P, 16], bf16)
            nc.gpsimd.memset(warm[:], 0.0)
            warm_in = dram.tile([P, 16], bf16, name="warm_in")
            warm_out = dram.tile([P, 16], bf16, name="warm_out",
                                 addr_space="Shared")
            nc.gpsimd.dma_start(out=warm_in[:], in_=warm[:])
            nc.gpsimd.collective_compute(
                "AllReduce",
                mybir.AluOpType.add,
                replica_groups=[list(range(NCORES))],
                ins=[warm_in.opt()],
                outs=[warm_out.opt()],
            )

            # ---- input DMA streams ----
            # k groups on sync, v groups + xT on scalar. Early groups are
            # split per chunk (and the very first chunk per half) so the
            # first matmuls' dependencies are small transfers.
            W0 = PH_BLOCKS[0] * P
            v_g = []
            k_g = [[] for _ in range(4)]
            for g in range(NVG):
                vt = vpool.tile([P, VG * D], bf16, name="vt", tag="v")
                if g == 0:
                    for s in range(VG):
                        if s == 0:
                            nc.scalar.dma_start(
                                out=vt[:, :HALF],
                                in_=vs_d.ap()[0][:, :HALF])
                            nc.scalar.dma_start(
                                out=vt[:, HALF:D],
                                in_=vs_d.ap()[0][:, HALF:D])
                        else:
                            nc.scalar.dma_start(
                                out=vt[:, s * D:(s + 1) * D],
                                in_=vs_d.ap()[0][:, s * D:(s + 1) * D])
                else:
                    nc.scalar.dma_start(out=vt[:], in_=vs_d.ap()[g])
                v_g.append(vt)
                # two k groups per v group to keep the streams paired
                for gg in (2 * g, 2 * g + 1):
                    kt = kpool.tile([P, KG * W0], bf16, name="k0t",
                                    tag="k0", bufs=2)
                    if gg < 4:
                        for s in range(KG):
                            nc.sync.dma_start(
                                out=kt[:, s * W0:(s + 1) * W0],
                                in_=ks_d[0].ap()[gg][:, s * W0:(s + 1) * W0])
                    else:
                        nc.sync.dma_start(out=kt[:], in_=ks_d[0].ap()[gg])
                    k_g[0].append(kt)
            for p in range(1, 4):
                wp = PH_BLOCKS[p] * P
                for g in range(NKG):
                    kt = kpool.tile([P, KG * wp], bf16,
                                    name=f"k{p}t", tag=f"k{p}", bufs=2)
                    nc.sync.dma_start(out=kt[:], in_=ks_d[p].ap()[g])
                    k_g[p].append(kt)
            xT_tiles = []
            for j in range(DB):
                xt = xpool.tile([P, T], bf16, name="xt", tag="x")
                nc.scalar.dma_start(out=xt[:], in_=xT_d.ap()[j])
                xT_tiles.append(xt)

            # ---- stage 2: kv partial, full-PSUM accumulation phases ----
            kvr = []           # AllReduced kv row-blocks (bf16, SBUF)
            for p in range(4):
                nb = PH_BLOCKS[p]
                wp = nb * P
                pst = [[ps.tile([P, HALF], f32,
                                name=f"kv{PH_J0[p] + b}_{h}", tag="ps")
                        for h in range(2)] for b in range(nb)]
                for c in range(NCH):
                    kt = k_g[p][c // KG]
                    vt = v_g[c // VG]
                    ko = (c % KG) * wp
                    vo = (c % VG) * D
                    for b in range(nb):
                        for h in range(2):
                            nc.tensor.matmul(
                                pst[b][h][:],
                                kt[:, ko + b * P:ko + (b + 1) * P],
                                vt[:, vo + h * HALF:vo + (h + 1) * HALF],
                                start=(c == 0), stop=(c == NCH - 1))
                # Drain: cast each row-block to bf16 with vector (h0) and
                # scalar (h1) in parallel, bounce to DRAM, AllReduce
                # while later phases compute.
                bounce_in = dram.tile([P, nb * D], bf16, name=f"bin{p}",
                                      tag=f"bin{p}")
                bounce_out = dram.tile([P, nb * D], bf16, name=f"bout{p}",
                                       tag=f"bout{p}", addr_space="Shared")
                for b in range(nb):
                    kvd = kvdp.tile([P, D], bf16, name=f"kvd{PH_J0[p] + b}",
                                    tag="kvd")
                    nc.vector.tensor_copy(out=kvd[:, :HALF],
                                          in_=pst[b][0][:])
                    nc.scalar.activation(kvd[:, HALF:], pst[b][1][:],
                                         ACT_COPY)
                    nc.gpsimd.dma_start(
                        out=bounce_in[:, b * D:(b + 1) * D], in_=kvd[:])
                nc.gpsimd.collective_compute(
                    "AllReduce",
                    mybir.AluOpType.add,
                    replica_groups=[list(range(NCORES))],
                    ins=[bounce_in.opt()],
                    outs=[bounce_out.opt()],
                )
                for b in range(nb):
                    kvj = kvrp.tile([P, D], bf16, name=f"kvr{PH_J0[p] + b}",
                                    tag="kvr")
                    nc.gpsimd.dma_start(
                        out=kvj[:], in_=bounce_out[:, b * D:(b + 1) * D])
                    kvr.append(kvj)

            # ---- stage 4: out = x @ kv, j-outer, last block deferred ----
            NJ = DB - 1        # row-blocks available before the last AR

            def s4_mm(po_t, i, h, j, start, stop):
                nc.tensor.matmul(
                    po_t[:],
                    xT_tiles[j][:, i * P:(i + 1) * P],
                    kvr[j][:, h * HALF:(h + 1) * HALF],
                    start=start, stop=stop)

            obs = [outp.tile([P, D], f32, name=f"ob{i}", tag="ob")
                   for i in range(TCH)]
            # h0 partials (j0-6), copied into the staging tiles.
            psA = [ps.tile([P, HALF], f32, name=f"pA{i}", tag="ps")
                   for i in range(TCH)]
            for j in range(NJ):
                for i in range(TCH):
                    s4_mm(psA[i], i, 0, j, j == 0, j == NJ - 1)
            for i in range(TCH):
                nc.vector.tensor_copy(out=obs[i][:, :HALF], in_=psA[i][:])
            # h1 partials (j0-6), likewise.
            psB = [ps.tile([P, HALF], f32, name=f"pB{i}", tag="ps")
                   for i in range(TCH)]
            for j in range(NJ):
                for i in range(TCH):
                    s4_mm(psB[i], i, 1, j, j == 0, j == NJ - 1)
            for i in range(TCH):
                if i % 2 == 0:
                    nc.vector.tensor_copy(out=obs[i][:, HALF:],
                                          in_=psB[i][:])
                else:
                    nc.scalar.activation(obs[i][:, HALF:], psB[i][:],
                                         ACT_COPY)
            # After the last AllReduce lands: 16 independent j7 matmuls,
            # added into the staging tiles (vector: h0, gpsimd: h1), then
            # one DMA per token chunk (split across sync and scalar).
            psD = []
            for i in range(TCH):
                pd0 = ps.tile([P, HALF], f32, name=f"pD0_{i}", tag="ps")
                s4_mm(pd0, i, 0, DB - 1, True, True)
                pd1 = ps.tile([P, HALF], f32, name=f"pD1_{i}", tag="ps")
                s4_mm(pd1, i, 1, DB - 1, True, True)
                psD.append((pd0, pd1))
            for i in range(TCH):
                pd0, pd1 = psD[i]
                nc.vector.tensor_tensor(
                    out=obs[i][:, :HALF], in0=pd0[:], in1=obs[i][:, :HALF],
                    op=mybir.AluOpType.add)
                nc.vector.tensor_tensor(
                    out=obs[i][:, HALF:], in0=pd1[:], in1=obs[i][:, HALF:],
                    op=mybir.AluOpType.add)
                eng = nc.sync if i % 2 == 0 else nc.scalar
                eng.dma_start(
                    out=out_d.ap()[i * P:(i + 1) * P, :], in_=obs[i][:])

    nc.compile()
    return nc


def _get_nc():
    if "nc" not in _CACHE:
        _CACHE["nc"] = _build_nc()
    return _CACHE["nc"]


def _group(a, grp):
    """[64*128, W] -> [64/grp, 128, grp*W]: pack row-chunks side by side."""
    w = a.shape[1]
    ng = NCH // grp
    return np.ascontiguousarray(
        a.reshape(ng, grp, P, w).transpose(0, 2, 1, 3).reshape(
            ng, P, grp * w))


def make_in_maps(inputs):
    import ml_dtypes

    bf16 = ml_dtypes.bfloat16
    x = np.asarray(inputs["x"], dtype=np.float32).reshape(B * S, D)
    keys = np.asarray(inputs["keys"], dtype=np.float32)
    vals = np.asarray(inputs["vals"], dtype=np.float32)

    col0 = [b * P for b in PH_J0] + [D]
    in_maps = []
    for c in range(NCORES):
        ksh = keys[c * KM:(c + 1) * KM].astype(bf16)
        m = {
            "vs": _group(vals[c * KM:(c + 1) * KM].astype(bf16), VG),
            "xT": np.ascontiguousarray(
                x[c * T:(c + 1) * T].T).astype(bf16).reshape(DB, P, T),
        }
        for p in range(4):
            m[f"ks{p}"] = _group(
                np.ascontiguousarray(ksh[:, col0[p]:col0[p + 1]]), KG)
        in_maps.append(m)
    return in_maps


def kernel(**inputs):
    from concourse.bass_utils import run_bass_kernel_spmd

    nc = _get_nc()
    in_maps = make_in_maps(inputs)
    res = run_bass_kernel_spmd(nc, in_maps, list(range(NCORES)))
    out = np.concatenate([res.results[c]["out"] for c in range(NCORES)],
                         axis=0)
    return out.reshape(B, S, D).astype(np.float32)


# revision 13
# speedup vs baseline: 1.0872x; 1.0872x over previous
"""Trainium2 Bass kernel for linear attention over external memory.

Computes out = x @ (keys^T @ vals) for
  x [4, 2048, 1024] f32, keys/vals [65536, 1024] f32.

Sharding across 8 NeuronCores: keys/vals sharded along the memory dim M
(8192 rows per core); each core computes a partial kv = keys_s^T @ vals_s,
AllReduces kv in bf16, then computes its token shard of x @ kv
(x sharded by token, 1024 rows per core).

All inputs are cast to bf16 on the host (halves DMA traffic, enables FWL
fast weight loads). x is transposed on the host. k/v chunks are packed
on the host into groups (v: 4 chunks, k: 2 chunks) so each input DMA
moves a large contiguous block — dma_start costs ~0.7us of sequencer
issue time each — and k (sync) / v+xT (scalar) streams issue on separate
queues. The first few groups are split into per-chunk DMAs so the PE can
start as soon as a small transfer lands.

Stage 2 accumulates kv fully in PSUM over all 64 k-chunks in 4 phases of
3/2/2/1 kv row-blocks (phase 0 is larger so its compute covers the input
DMA ramp; the last phase is small so the final AllReduce is small).
Phase results are cast to bf16 (vector+scalar halves in parallel) and
AllReduced while later phases compute; all collective-path ops sit on
the gpsimd queue in program order.

Stage 4 (out = x @ kv) runs j-outer with the last row-block deferred:
j0-6 partials for both column halves are copied into the output staging
tiles (freeing all PSUM banks) while the last AllReduce is in flight;
once it lands, 16 independent single-matmul j7 groups stream through the
PE back-to-back and vector/gpsimd add them into the staging tiles.
"""

import numpy as np

# Problem shapes (hardcoded per contract).
B, S, D = 4, 2048, 1024
M = 65536
NCORES = 8
P = 128
T = (B * S) // NCORES          # 1024 tokens per core
KM = M // NCORES               # 8192 memory rows per core
NCH = KM // P                  # 64 k-chunks
DB = D // P                    # 8 d / kv-row blocks
TCH = T // P                   # 8 token chunks
HALF = D // 2                  # 512
VG = 4                         # chunks per v DMA group
KG0 = 2                        # chunks per k DMA group (phase 0)
KG = 4                         # chunks per k DMA group (phases 1-3)
NVG = NCH // VG                # 16 v groups
NKG0 = NCH // KG0              # 32 k0 groups
NKG = NCH // KG                # 16 k groups

# Stage-2 phases: number of kv row-blocks finished per phase.
PH_BLOCKS = [3, 2, 2, 1]
PH_J0 = [0, 3, 5, 7]           # first row-block of each phase

_CACHE = {}


def _build_nc():
    import concourse.bacc as bacc
    import concourse.tile as tile
    from concourse import mybir

    f32 = mybir.dt.float32
    bf16 = mybir.dt.bfloat16
    ACT_COPY = mybir.ActivationFunctionType.Copy

    nc = bacc.Bacc("TRN2", target_bir_lowering=False, debug=False,
                   num_devices=NCORES)

    # k per phase, v, and xT arrive pre-grouped from the host.
    ks_d = [nc.dram_tensor(
        f"ks{p}",
        [NKG0 if p == 0 else NKG, P,
         (KG0 if p == 0 else KG) * PH_BLOCKS[p] * P],
        bf16, kind="ExternalInput")
            for p in range(4)]
    vs_d = nc.dram_tensor("vs", [NVG, P, VG * D], bf16, kind="ExternalInput")
    xT_d = nc.dram_tensor("xT", [DB, P, T], bf16, kind="ExternalInput")
    out_d = nc.dram_tensor("out", [T, D], f32, kind="ExternalOutput")

    with tile.TileContext(nc) as tc:
        with (
            tc.tile_pool(name="const", bufs=1) as const,
            tc.tile_pool(name="vpool", bufs=NVG) as vpool,
            tc.tile_pool(name="kpool", bufs=2) as kpool,
            tc.tile_pool(name="xpool", bufs=DB) as xpool,
            tc.tile_pool(name="kvdp", bufs=1) as kvdp,
            tc.tile_pool(name="kvrp", bufs=DB) as kvrp,
            tc.tile_pool(name="outp", bufs=TCH) as outp,
            tc.tile_pool(name="ps", bufs=8, space="PSUM") as ps,
            tc.tile_pool(name="dram", bufs=10, space="DRAM") as dram,
        ):
            # Warm-up collective: arms the ncfw collective stream so the
            # first real AllReduce trigger doesn't pay the wake-up.
            warm = const.tile([P, 16], bf16)
            nc.gpsimd.memset(warm[:], 0.0)
            warm_in = dram.tile([P, 16], bf16, name="warm_in")
            warm_out = dram.tile([P, 16], bf16, name="warm_out",
                                 addr_space="Shared")
            nc.gpsimd.dma_start(out=warm_in[:], in_=warm[:])
            nc.gpsimd.collective_compute(
                "AllReduce",
                mybir.AluOpType.add,
                replica_groups=[list(range(NCORES))],
                ins=[warm_in.opt()],
                outs=[warm_out.opt()],
            )

            # ---- input DMA streams ----
            # k groups on sync, v groups + xT on scalar. Early groups are
            # split per chunk (and the very first chunk per half) so the
            # first matmuls' dependencies are small transfers.
            W0 = PH_BLOCKS[0] * P
            v_g = []
            k_g = [[] for _ in range(4)]
            for g in range(NVG):
                vt = vpool.tile([P, VG * D], bf16, name="vt", tag="v")
                if g == 0:
                    for s in range(VG):
                        if s == 0:
                            nc.scalar.dma_start(
                                out=vt[:, :HALF],
                                in_=vs_d.ap()[0][:, :HALF])
                            nc.scalar.dma_start(
                                out=vt[:, HALF:D],
                                in_=vs_d.ap()[0][:, HALF:D])
                        else:
                            nc.scalar.dma_start(
                                out=vt[:, s * D:(s + 1) * D],
                                in_=vs_d.ap()[0][:, s * D:(s + 1) * D])
                else:
                    nc.scalar.dma_start(out=vt[:], in_=vs_d.ap()[g])
                v_g.append(vt)
                # two k groups per v group to keep the streams paired
                for gg in (2 * g, 2 * g + 1):
                    kt = kpool.tile([P, KG0 * W0], bf16, name="k0t",
                                    tag="k0", bufs=2)
                    if gg < 4:
                        for s in range(KG0):
                            nc.sync.dma_start(
                                out=kt[:, s * W0:(s + 1) * W0],
                                in_=ks_d[0].ap()[gg][:, s * W0:(s + 1) * W0])
                    else:
                        nc.sync.dma_start(out=kt[:], in_=ks_d[0].ap()[gg])
                    k_g[0].append(kt)
            for p in range(1, 4):
                wp = PH_BLOCKS[p] * P
                for g in range(NKG):
                    kt = kpool.tile([P, KG * wp], bf16,
                                    name=f"k{p}t", tag=f"k{p}", bufs=2)
                    nc.sync.dma_start(out=kt[:], in_=ks_d[p].ap()[g])
                    k_g[p].append(kt)
            xT_tiles = []
            for j in range(DB):
                xt = xpool.tile([P, T], bf16, name="xt", tag="x")
                nc.scalar.dma_start(out=xt[:], in_=xT_d.ap()[j])
                xT_tiles.append(xt)

            # ---- stage 2: kv partial, full-PSUM accumulation phases ----
            kvr = []           # AllReduced kv row-blocks (bf16, SBUF)
            for p in range(4):
                nb = PH_BLOCKS[p]
                wp = nb * P
                pst = [[ps.tile([P, HALF], f32,
                                name=f"kv{PH_J0[p] + b}_{h}", tag="ps")
                        for h in range(2)] for b in range(nb)]
                kg = KG0 if p == 0 else KG
                for c in range(NCH):
                    kt = k_g[p][c // kg]
                    vt = v_g[c // VG]
                    ko = (c % kg) * wp
                    vo = (c % VG) * D
                    for b in range(nb):
                        for h in range(2):
                            nc.tensor.matmul(
                                pst[b][h][:],
                                kt[:, ko + b * P:ko + (b + 1) * P],
                                vt[:, vo + h * HALF:vo + (h + 1) * HALF],
                                start=(c == 0), stop=(c == NCH - 1))
                # Drain: cast each row-block to bf16 with vector (h0) and
                # scalar (h1) in parallel, bounce to DRAM, AllReduce
                # while later phases compute.
                bounce_in = dram.tile([P, nb * D], bf16, name=f"bin{p}",
                                      tag=f"bin{p}")
                bounce_out = dram.tile([P, nb * D], bf16, name=f"bout{p}",
                                       tag=f"bout{p}", addr_space="Shared")
                for b in range(nb):
                    kvd = kvdp.tile([P, D], bf16, name=f"kvd{PH_J0[p] + b}",
                                    tag="kvd")
                    nc.vector.tensor_copy(out=kvd[:, :HALF],
                                          in_=pst[b][0][:])
                    nc.scalar.activation(kvd[:, HALF:], pst[b][1][:],
                                         ACT_COPY)
                    nc.gpsimd.dma_start(
                        out=bounce_in[:, b * D:(b + 1) * D], in_=kvd[:])
                nc.gpsimd.collective_compute(
                    "AllReduce",
                    mybir.AluOpType.add,
                    replica_groups=[list(range(NCORES))],
                    ins=[bounce_in.opt()],
                    outs=[bounce_out.opt()],
                )
                for b in range(nb):
                    kvj = kvrp.tile([P, D], bf16, name=f"kvr{PH_J0[p] + b}",
                                    tag="kvr")
                    nc.gpsimd.dma_start(
                        out=kvj[:], in_=bounce_out[:, b * D:(b + 1) * D])
                    kvr.append(kvj)

            # ---- stage 4: out = x @ kv, j-outer, last block deferred ----
            NJ = DB - 1        # row-blocks available before the last AR

            def s4_mm(po_t, i, h, j, start, stop):
                nc.tensor.matmul(
                    po_t[:],
                    xT_tiles[j][:, i * P:(i + 1) * P],
                    kvr[j][:, h * HALF:(h + 1) * HALF],
                    start=start, stop=stop)

            obs = [outp.tile([P, D], f32, name=f"ob{i}", tag="ob")
                   for i in range(TCH)]
            # h0 partials (j0-6), copied into the staging tiles.
            psA = [ps.tile([P, HALF], f32, name=f"pA{i}", tag="ps")
                   for i in range(TCH)]
            for j in range(NJ):
                for i in range(TCH):
                    s4_mm(psA[i], i, 0, j, j == 0, j == NJ - 1)
            for i in range(TCH):
                nc.vector.tensor_copy(out=obs[i][:, :HALF], in_=psA[i][:])
            # h1 partials (j0-6), likewise.
            psB = [ps.tile([P, HALF], f32, name=f"pB{i}", tag="ps")
                   for i in range(TCH)]
            for j in range(NJ):
                for i in range(TCH):
                    s4_mm(psB[i], i, 1, j, j == 0, j == NJ - 1)
            for i in range(TCH):
                if i % 2 == 0:
                    nc.vector.tensor_copy(out=obs[i][:, HALF:],
                                          in_=psB[i][:])
                else:
                    nc.scalar.activation(obs[i][:, HALF:], psB[i][:],
                                         ACT_COPY)
            # After the last AllReduce lands: 16 independent j7 matmuls,
            # added into the staging tiles (vector: h0, gpsimd: h1), then
            # one DMA per token chunk (split across sync and scalar).
            psD = []
            for i in range(TCH):
                pd0 = ps.tile([P, HALF], f32, name=f"pD0_{i}", tag="ps")
                s4_mm(pd0, i, 0, DB - 1, True, True)
                pd1 = ps.tile([P, HALF], f32, name=f"pD1_{i}", tag="ps")
                s4_mm(pd1, i, 1, DB - 1, True, True)
                psD.append((pd0, pd1))
            for i in range(TCH):
                pd0, pd1 = psD[i]
                nc.vector.tensor_tensor(
                    out=obs[i][:, :HALF], in0=pd0[:], in1=obs[i][:, :HALF],
                    op=mybir.AluOpType.add)
                nc.vector.tensor_tensor(
                    out=obs[i][:, HALF:], in0=pd1[:], in1=obs[i][:, HALF:],
                    op=mybir.AluOpType.add)
                eng = nc.sync if i % 2 == 0 else nc.scalar
                eng.dma_start(
                    out=out_d.ap()[i * P:(i + 1) * P, :], in_=obs[i][:])

    nc.compile()
    return nc


def _get_nc():
    if "nc" not in _CACHE:
        _CACHE["nc"] = _build_nc()
    return _CACHE["nc"]


def _group(a, grp):
    """[64*128, W] -> [64/grp, 128, grp*W]: pack row-chunks side by side."""
    w = a.shape[1]
    ng = NCH // grp
    return np.ascontiguousarray(
        a.reshape(ng, grp, P, w).transpose(0, 2, 1, 3).reshape(
            ng, P, grp * w))


def make_in_maps(inputs):
    import ml_dtypes

    bf16 = ml_dtypes.bfloat16
    x = np.asarray(inputs["x"], dtype=np.float32).reshape(B * S, D)
    keys = np.asarray(inputs["keys"], dtype=np.float32)
    vals = np.asarray(inputs["vals"], dtype=np.float32)

    col0 = [b * P for b in PH_J0] + [D]
    in_maps = []
    for c in range(NCORES):
        ksh = keys[c * KM:(c + 1) * KM].astype(bf16)
        m = {
            "vs": _group(vals[c * KM:(c + 1) * KM].astype(bf16), VG),
            "xT": np.ascontiguousarray(
                x[c * T:(c + 1) * T].T).astype(bf16).reshape(DB, P, T),
        }
        for p in range(4):
            m[f"ks{p}"] = _group(
                np.ascontiguousarray(ksh[:, col0[p]:col0[p + 1]]),
                KG0 if p == 0 else KG)
        in_maps.append(m)
    return in_maps


def kernel(**inputs):
    from concourse.bass_utils import run_bass_kernel_spmd

    nc = _get_nc()
    in_maps = make_in_maps(inputs)
    res = run_bass_kernel_spmd(nc, in_maps, list(range(NCORES)))
    out = np.concatenate([res.results[c]["out"] for c in range(NCORES)],
                         axis=0)
    return out.reshape(B, S, D).astype(np.float32)


# revision 14
# speedup vs baseline: 1.0887x; 1.0014x over previous
"""Trainium2 Bass kernel for linear attention over external memory.

Computes out = x @ (keys^T @ vals) for
  x [4, 2048, 1024] f32, keys/vals [65536, 1024] f32.

Sharding across 8 NeuronCores: keys/vals sharded along the memory dim M
(8192 rows per core); each core computes a partial kv = keys_s^T @ vals_s,
AllReduces kv in bf16, then computes its token shard of x @ kv
(x sharded by token, 1024 rows per core).

All inputs are cast to bf16 on the host (halves DMA traffic, enables FWL
fast weight loads). x is transposed on the host. k/v chunks are packed
on the host into groups (v: 4 chunks, k: 2 chunks) so each input DMA
moves a large contiguous block — dma_start costs ~0.7us of sequencer
issue time each — and k (sync) / v+xT (scalar) streams issue on separate
queues. The first few groups are split into per-chunk DMAs so the PE can
start as soon as a small transfer lands.

Stage 2 accumulates kv fully in PSUM over all 64 k-chunks in 4 phases of
3/2/2/1 kv row-blocks (phase 0 is larger so its compute covers the input
DMA ramp; the last phase is small so the final AllReduce is small).
Phase results are cast to bf16 (vector+scalar halves in parallel) and
AllReduced while later phases compute; all collective-path ops sit on
the gpsimd queue in program order.

Stage 4 (out = x @ kv) runs j-outer with the last row-block deferred:
j0-6 partials for both column halves are copied into the output staging
tiles (freeing all PSUM banks) while the last AllReduce is in flight;
once it lands, 16 independent single-matmul j7 groups stream through the
PE back-to-back and vector/gpsimd add them into the staging tiles.
"""

import numpy as np

# Problem shapes (hardcoded per contract).
B, S, D = 4, 2048, 1024
M = 65536
NCORES = 8
P = 128
T = (B * S) // NCORES          # 1024 tokens per core
KM = M // NCORES               # 8192 memory rows per core
NCH = KM // P                  # 64 k-chunks
DB = D // P                    # 8 d / kv-row blocks
TCH = T // P                   # 8 token chunks
HALF = D // 2                  # 512
VG = 4                         # chunks per v DMA group
KG0 = 2                        # chunks per k DMA group (phase 0)
KG = 4                         # chunks per k DMA group (phases 1-3)
NVG = NCH // VG                # 16 v groups
NKG0 = NCH // KG0              # 32 k0 groups
NKG = NCH // KG                # 16 k groups

# Stage-2 phases: number of kv row-blocks finished per phase.
PH_BLOCKS = [3, 2, 2, 1]
PH_J0 = [0, 3, 5, 7]           # first row-block of each phase

_CACHE = {}


def _build_nc():
    import concourse.bacc as bacc
    import concourse.tile as tile
    from concourse import mybir

    f32 = mybir.dt.float32
    bf16 = mybir.dt.bfloat16
    ACT_COPY = mybir.ActivationFunctionType.Copy

    nc = bacc.Bacc("TRN2", target_bir_lowering=False, debug=False,
                   num_devices=NCORES)

    # k per phase, v, and xT arrive pre-grouped from the host.
    ks_d = [nc.dram_tensor(
        f"ks{p}",
        [NKG0 if p == 0 else NKG, P,
         (KG0 if p == 0 else KG) * PH_BLOCKS[p] * P],
        bf16, kind="ExternalInput")
            for p in range(4)]
    vs_d = nc.dram_tensor("vs", [NVG, P, VG * D], bf16, kind="ExternalInput")
    xT_d = nc.dram_tensor("xT", [DB, P, T], bf16, kind="ExternalInput")
    out_d = nc.dram_tensor("out", [T, D], f32, kind="ExternalOutput")

    with tile.TileContext(nc) as tc:
        with (
            tc.tile_pool(name="const", bufs=1) as const,
            tc.tile_pool(name="vpool", bufs=NVG) as vpool,
            tc.tile_pool(name="kpool", bufs=2) as kpool,
            tc.tile_pool(name="xpool", bufs=DB) as xpool,
            tc.tile_pool(name="kvdp", bufs=1) as kvdp,
            tc.tile_pool(name="kvrp", bufs=DB) as kvrp,
            tc.tile_pool(name="outp", bufs=TCH) as outp,
            tc.tile_pool(name="ps", bufs=8, space="PSUM") as ps,
            tc.tile_pool(name="dram", bufs=10, space="DRAM") as dram,
        ):
            # Warm-up collective: arms the ncfw collective stream so the
            # first real AllReduce trigger doesn't pay the wake-up.
            warm = const.tile([P, 16], bf16)
            nc.gpsimd.memset(warm[:], 0.0)
            warm_in = dram.tile([P, 16], bf16, name="warm_in")
            warm_out = dram.tile([P, 16], bf16, name="warm_out",
                                 addr_space="Shared")
            nc.gpsimd.dma_start(out=warm_in[:], in_=warm[:])
            nc.gpsimd.collective_compute(
                "AllReduce",
                mybir.AluOpType.add,
                replica_groups=[list(range(NCORES))],
                ins=[warm_in.opt()],
                outs=[warm_out.opt()],
            )

            # ---- input DMA streams ----
            # k groups on sync, v groups + xT on scalar. Early groups are
            # split per chunk (and the very first chunk per half) so the
            # first matmuls' dependencies are small transfers.
            W0 = PH_BLOCKS[0] * P
            v_g = []
            k_g = [[] for _ in range(4)]
            for g in range(NVG):
                vt = vpool.tile([P, VG * D], bf16, name="vt", tag="v")
                if g == 0:
                    for s in range(VG):
                        if s == 0:
                            nc.scalar.dma_start(
                                out=vt[:, :HALF],
                                in_=vs_d.ap()[0][:, :HALF])
                            nc.scalar.dma_start(
                                out=vt[:, HALF:D],
                                in_=vs_d.ap()[0][:, HALF:D])
                        else:
                            nc.scalar.dma_start(
                                out=vt[:, s * D:(s + 1) * D],
                                in_=vs_d.ap()[0][:, s * D:(s + 1) * D])
                else:
                    nc.scalar.dma_start(out=vt[:], in_=vs_d.ap()[g])
                v_g.append(vt)
                # two k groups per v group to keep the streams paired
                for gg in (2 * g, 2 * g + 1):
                    kt = kpool.tile([P, KG0 * W0], bf16, name="k0t",
                                    tag="k0", bufs=2)
                    if gg < 4:
                        for s in range(KG0):
                            nc.sync.dma_start(
                                out=kt[:, s * W0:(s + 1) * W0],
                                in_=ks_d[0].ap()[gg][:, s * W0:(s + 1) * W0])
                    else:
                        nc.sync.dma_start(out=kt[:], in_=ks_d[0].ap()[gg])
                    k_g[0].append(kt)
            for p in range(1, 4):
                wp = PH_BLOCKS[p] * P
                for g in range(NKG):
                    kt = kpool.tile([P, KG * wp], bf16,
                                    name=f"k{p}t", tag=f"k{p}", bufs=2)
                    nc.sync.dma_start(out=kt[:], in_=ks_d[p].ap()[g])
                    k_g[p].append(kt)
            xT_tiles = []
            for j in range(DB):
                xt = xpool.tile([P, T], bf16, name="xt", tag="x")
                nc.scalar.dma_start(out=xt[:], in_=xT_d.ap()[j])
                xT_tiles.append(xt)

            # ---- stage 2: kv partial, full-PSUM accumulation phases ----
            kvr = []           # AllReduced kv row-blocks (bf16, SBUF)
            for p in range(4):
                nb = PH_BLOCKS[p]
                wp = nb * P
                pst = [[ps.tile([P, HALF], f32,
                                name=f"kv{PH_J0[p] + b}_{h}", tag="ps")
                        for h in range(2)] for b in range(nb)]
                kg = KG0 if p == 0 else KG
                for c in range(NCH):
                    kt = k_g[p][c // kg]
                    vt = v_g[c // VG]
                    ko = (c % kg) * wp
                    vo = (c % VG) * D
                    for b in range(nb):
                        for h in range(2):
                            nc.tensor.matmul(
                                pst[b][h][:],
                                kt[:, ko + b * P:ko + (b + 1) * P],
                                vt[:, vo + h * HALF:vo + (h + 1) * HALF],
                                start=(c == 0), stop=(c == NCH - 1))
                # Drain: cast each row-block to bf16 with vector (h0) and
                # scalar (h1) in parallel, bounce to DRAM, AllReduce
                # while later phases compute.
                bounce_in = dram.tile([P, nb * D], bf16, name=f"bin{p}",
                                      tag=f"bin{p}")
                bounce_out = dram.tile([P, nb * D], bf16, name=f"bout{p}",
                                       tag=f"bout{p}", addr_space="Shared")
                for b in range(nb):
                    kvd = kvdp.tile([P, D], bf16, name=f"kvd{PH_J0[p] + b}",
                                    tag="kvd")
                    nc.vector.tensor_copy(out=kvd[:, :HALF],
                                          in_=pst[b][0][:])
                    nc.scalar.activation(kvd[:, HALF:], pst[b][1][:],
                                         ACT_COPY)
                    nc.gpsimd.dma_start(
                        out=bounce_in[:, b * D:(b + 1) * D], in_=kvd[:])
                nc.gpsimd.collective_compute(
                    "AllReduce",
                    mybir.AluOpType.add,
                    replica_groups=[list(range(NCORES))],
                    ins=[bounce_in.opt()],
                    outs=[bounce_out.opt()],
                )
                for b in range(nb):
                    kvj = kvrp.tile([P, D], bf16, name=f"kvr{PH_J0[p] + b}",
                                    tag="kvr")
                    nc.gpsimd.dma_start(
                        out=kvj[:], in_=bounce_out[:, b * D:(b + 1) * D])
                    kvr.append(kvj)

            # ---- stage 4: out = x @ kv, j-outer, last block deferred ----
            NJ = DB - 1        # row-blocks available before the last AR

            def s4_mm(po_t, i, h, j, start, stop):
                nc.tensor.matmul(
                    po_t[:],
                    xT_tiles[j][:, i * P:(i + 1) * P],
                    kvr[j][:, h * HALF:(h + 1) * HALF],
                    start=start, stop=stop)

            obs = [outp.tile([P, D], f32, name=f"ob{i}", tag="ob")
                   for i in range(TCH)]
            # h0 partials (j0-6), copied into the staging tiles.
            psA = [ps.tile([P, HALF], f32, name=f"pA{i}", tag="ps")
                   for i in range(TCH)]
            for j in range(NJ):
                for i in range(TCH):
                    s4_mm(psA[i], i, 0, j, j == 0, j == NJ - 1)
            for i in range(TCH):
                nc.vector.tensor_copy(out=obs[i][:, :HALF], in_=psA[i][:])
            # h1 partials (j0-6), likewise.
            psB = [ps.tile([P, HALF], f32, name=f"pB{i}", tag="ps")
                   for i in range(TCH)]
            for j in range(NJ):
                for i in range(TCH):
                    s4_mm(psB[i], i, 1, j, j == 0, j == NJ - 1)
            for i in range(TCH):
                if i % 2 == 0:
                    nc.vector.tensor_copy(out=obs[i][:, HALF:],
                                          in_=psB[i][:])
                else:
                    nc.scalar.activation(obs[i][:, HALF:], psB[i][:],
                                         ACT_COPY)
            # After the last AllReduce lands: 16 independent j7 matmuls,
            # added into the staging tiles (vector: h0, gpsimd: h1), then
            # one DMA per token chunk (split across sync and scalar).
            psD = []
            for i in range(TCH):
                pd0 = ps.tile([P, HALF], f32, name=f"pD0_{i}", tag="ps")
                s4_mm(pd0, i, 0, DB - 1, True, True)
                pd1 = ps.tile([P, HALF], f32, name=f"pD1_{i}", tag="ps")
                s4_mm(pd1, i, 1, DB - 1, True, True)
                psD.append((pd0, pd1))
            for i in range(TCH):
                pd0, pd1 = psD[i]
                nc.vector.tensor_tensor(
                    out=obs[i][:, :HALF], in0=pd0[:], in1=obs[i][:, :HALF],
                    op=mybir.AluOpType.add)
                nc.sync.dma_start(
                    out=out_d.ap()[i * P:(i + 1) * P, :HALF],
                    in_=obs[i][:, :HALF])
                nc.vector.tensor_tensor(
                    out=obs[i][:, HALF:], in0=pd1[:], in1=obs[i][:, HALF:],
                    op=mybir.AluOpType.add)
                nc.scalar.dma_start(
                    out=out_d.ap()[i * P:(i + 1) * P, HALF:],
                    in_=obs[i][:, HALF:])

    nc.compile()
    return nc


def _get_nc():
    if "nc" not in _CACHE:
        _CACHE["nc"] = _build_nc()
    return _CACHE["nc"]


def _group(a, grp):
    """[64*128, W] -> [64/grp, 128, grp*W]: pack row-chunks side by side."""
    w = a.shape[1]
    ng = NCH // grp
    return np.ascontiguousarray(
        a.reshape(ng, grp, P, w).transpose(0, 2, 1, 3).reshape(
            ng, P, grp * w))


def make_in_maps(inputs):
    import ml_dtypes

    bf16 = ml_dtypes.bfloat16
    x = np.asarray(inputs["x"], dtype=np.float32).reshape(B * S, D)
    keys = np.asarray(inputs["keys"], dtype=np.float32)
    vals = np.asarray(inputs["vals"], dtype=np.float32)

    col0 = [b * P for b in PH_J0] + [D]
    in_maps = []
    for c in range(NCORES):
        ksh = keys[c * KM:(c + 1) * KM].astype(bf16)
        m = {
            "vs": _group(vals[c * KM:(c + 1) * KM].astype(bf16), VG),
            "xT": np.ascontiguousarray(
                x[c * T:(c + 1) * T].T).astype(bf16).reshape(DB, P, T),
        }
        for p in range(4):
            m[f"ks{p}"] = _group(
                np.ascontiguousarray(ksh[:, col0[p]:col0[p + 1]]),
                KG0 if p == 0 else KG)
        in_maps.append(m)
    return in_maps


def kernel(**inputs):
    from concourse.bass_utils import run_bass_kernel_spmd

    nc = _get_nc()
    in_maps = make_in_maps(inputs)
    res = run_bass_kernel_spmd(nc, in_maps, list(range(NCORES)))
    out = np.concatenate([res.results[c]["out"] for c in range(NCORES)],
                         axis=0)
    return out.reshape(B, S, D).astype(np.float32)


# revision 15
# speedup vs baseline: 1.1333x; 1.0410x over previous
"""Trainium2 Bass kernel for linear attention over external memory.

Computes out = x @ (keys^T @ vals) for
  x [4, 2048, 1024] f32, keys/vals [65536, 1024] f32.

Sharding across 8 NeuronCores: keys/vals sharded along the memory dim M
(8192 rows per core); each core computes a partial kv = keys_s^T @ vals_s,
AllReduces kv in bf16, then computes its token shard of x @ kv
(x sharded by token, 1024 rows per core).

All inputs are cast to bf16 on the host (halves DMA traffic vs f32 and
enables FWL fast weight loads, which hides LDWEIGHTS under the matmul
stream). x is transposed on the host so no on-device PE transposes are
needed.

Stage 2 accumulates kv fully in PSUM over all 64 k-chunks in 4 phases of
3/2/2/1 kv row-blocks. Phase 0 is larger so its compute (~100us) covers
the input-DMA ramp; the last phase is small so the final AllReduce is
small. Phase results are cast to bf16 (vector h0 + scalar h1 in
parallel) and AllReduced while later phases compute; all collective-path
ops (bounce writes, triggers, result reads) sit on the gpsimd queue in
program order, which serializes the ARs naturally without delaying
other queues.

Stage 4 (out = x @ kv) runs j-outer with the last row-block deferred:
h0 column-half partials (j0-6) are stashed to SBUF to free PSUM for the
h1 half, keeping the PE busy for the ~25us the last AllReduce needs;
after it lands, short j7 passes finalize both halves.
"""

import numpy as np

# Problem shapes (hardcoded per contract).
B, S, D = 4, 2048, 1024
M = 65536
NCORES = 8
P = 128
T = (B * S) // NCORES          # 1024 tokens per core
KM = M // NCORES               # 8192 memory rows per core
NCH = KM // P                  # 64 k-chunks
DB = D // P                    # 8 d / kv-row blocks
TCH = T // P                   # 8 token chunks
HALF = D // 2                  # 512

# Stage-2 phases: number of kv row-blocks finished per phase.
PH_BLOCKS = [3, 2, 2, 1]
PH_J0 = [0, 3, 5, 7]           # first row-block of each phase
NDEF = 1                       # row-blocks deferred in stage 4 (= last AR)

_CACHE = {}


def _build_nc():
    import concourse.bacc as bacc
    import concourse.tile as tile
    from concourse import mybir

    f32 = mybir.dt.float32
    bf16 = mybir.dt.bfloat16
    ACT_COPY = mybir.ActivationFunctionType.Copy

    nc = bacc.Bacc("TRN2", target_bir_lowering=False, debug=False,
                   num_devices=NCORES)

    ks_d = [nc.dram_tensor(f"ks{p}", [KM, PH_BLOCKS[p] * P], bf16,
                           kind="ExternalInput")
            for p in range(4)]
    vs_d = nc.dram_tensor("vs", [KM, D], bf16, kind="ExternalInput")
    xT_d = nc.dram_tensor("xT", [D, T], bf16, kind="ExternalInput")
    out_d = nc.dram_tensor("out", [T, D], f32, kind="ExternalOutput")

    ks_r = [ks_d[p].ap().rearrange("(c p) n -> c p n", p=P) for p in range(4)]
    vs_r = vs_d.ap().rearrange("(c p) n -> c p n", p=P)     # [64, 128, 1024]
    xT_r = xT_d.ap().rearrange("(c p) n -> c p n", p=P)     # [8, 128, 1024]

    with tile.TileContext(nc) as tc:
        with (
            tc.tile_pool(name="const", bufs=1) as const,
            tc.tile_pool(name="vpool", bufs=NCH) as vpool,
            tc.tile_pool(name="kpool", bufs=5) as kpool,
            tc.tile_pool(name="xpool", bufs=DB) as xpool,
            tc.tile_pool(name="kvdp", bufs=2) as kvdp,
            tc.tile_pool(name="kvrp", bufs=DB) as kvrp,
            tc.tile_pool(name="stashp", bufs=TCH) as stashp,
            tc.tile_pool(name="outp", bufs=2) as outp,
            tc.tile_pool(name="ps", bufs=8, space="PSUM") as ps,
            tc.tile_pool(name="dram", bufs=10, space="DRAM") as dram,
        ):
            # Warm-up collective: arms the ncfw collective stream so the
            # first real AllReduce trigger doesn't pay the wake-up.
            warm = const.tile([P, 16], bf16)
            nc.gpsimd.memset(warm[:], 0.0)
            warm_in = dram.tile([P, 16], bf16, name="warm_in")
            warm_out = dram.tile([P, 16], bf16, name="warm_out",
                                 addr_space="Shared")
            nc.gpsimd.dma_start(out=warm_in[:], in_=warm[:])
            nc.gpsimd.collective_compute(
                "AllReduce",
                mybir.AluOpType.add,
                replica_groups=[list(range(NCORES))],
                ins=[warm_in.opt()],
                outs=[warm_out.opt()],
            )

            # ---- input DMA streams ----
            # Phase-0 k chunks and v chunks interleaved so stage-2 can
            # start immediately; later phases' k chunks follow. The very
            # first v chunk is split in half so the first matmul's
            # dependency is a minimal transfer.
            v_tiles = []
            k_tiles = [[] for _ in range(4)]
            for c in range(NCH):
                kt = kpool.tile([P, PH_BLOCKS[0] * P], bf16, name="k0t",
                                tag="k0", bufs=5)
                nc.sync.dma_start(out=kt[:], in_=ks_r[0][c])
                vt = vpool.tile([P, D], bf16, name="vt", tag="v")
                if c == 0:
                    nc.sync.dma_start(out=vt[:, :HALF],
                                      in_=vs_r[c][:, :HALF])
                    nc.sync.dma_start(out=vt[:, HALF:],
                                      in_=vs_r[c][:, HALF:])
                else:
                    nc.sync.dma_start(out=vt[:], in_=vs_r[c])
                k_tiles[0].append(kt)
                v_tiles.append(vt)
            for p in range(1, 4):
                for c in range(NCH):
                    kt = kpool.tile([P, PH_BLOCKS[p] * P], bf16,
                                    name=f"k{p}t", tag=f"k{p}", bufs=5)
                    nc.sync.dma_start(out=kt[:], in_=ks_r[p][c])
                    k_tiles[p].append(kt)
            # x^T tiles on the scalar queue (doesn't contend with k/v).
            xT_tiles = []
            for j in range(DB):
                xt = xpool.tile([P, T], bf16, name="xt", tag="x")
                nc.scalar.dma_start(out=xt[:], in_=xT_r[j])
                xT_tiles.append(xt)

            # ---- stage 2: kv partial, full-PSUM accumulation phases ----
            kvr = []           # AllReduced kv row-blocks (bf16, SBUF)
            for p in range(4):
                nb = PH_BLOCKS[p]
                pst = [[ps.tile([P, HALF], f32,
                                name=f"kv{PH_J0[p] + b}_{h}", tag="ps")
                        for h in range(2)] for b in range(nb)]
                for c in range(NCH):
                    kt = k_tiles[p][c]
                    vt = v_tiles[c]
                    for b in range(nb):
                        for h in range(2):
                            nc.tensor.matmul(
                                pst[b][h][:],
                                kt[:, b * P:(b + 1) * P],
                                vt[:, h * HALF:(h + 1) * HALF],
                                start=(c == 0), stop=(c == NCH - 1))
                # Drain: cast each row-block to bf16 with vector (h0) and
                # scalar (h1) in parallel, bounce to DRAM, AllReduce
                # while later phases compute.
                bounce_in = dram.tile([P, nb * D], bf16, name=f"bin{p}",
                                      tag=f"bin{p}")
                bounce_out = dram.tile([P, nb * D], bf16, name=f"bout{p}",
                                       tag=f"bout{p}", addr_space="Shared")
                for b in range(nb):
                    kvd = kvdp.tile([P, D], bf16, name=f"kvd{PH_J0[p] + b}",
                                    tag="kvd")
                    nc.vector.tensor_copy(out=kvd[:, :HALF],
                                          in_=pst[b][0][:])
                    nc.scalar.activation(kvd[:, HALF:], pst[b][1][:],
                                         ACT_COPY)
                    nc.gpsimd.dma_start(
                        out=bounce_in[:, b * D:(b + 1) * D], in_=kvd[:])
                nc.gpsimd.collective_compute(
                    "AllReduce",
                    mybir.AluOpType.add,
                    replica_groups=[list(range(NCORES))],
                    ins=[bounce_in.opt()],
                    outs=[bounce_out.opt()],
                )
                for b in range(nb):
                    kvj = kvrp.tile([P, D], bf16, name=f"kvr{PH_J0[p] + b}",
                                    tag="kvr")
                    nc.gpsimd.dma_start(
                        out=kvj[:], in_=bounce_out[:, b * D:(b + 1) * D])
                    kvr.append(kvj)

            # ---- stage 4: out = x @ kv, j-outer, last block deferred ----
            NJ = DB - NDEF     # row-blocks available before the last AR

            def s4_mm(po_t, i, h, j, start, stop):
                nc.tensor.matmul(
                    po_t[:],
                    xT_tiles[j][:, i * P:(i + 1) * P],
                    kvr[j][:, h * HALF:(h + 1) * HALF],
                    start=start, stop=stop)

            # h0 partials (j0-6) for all 8 chunks, stashed to SBUF.
            psA = [ps.tile([P, HALF], f32, name=f"pA{i}", tag="ps")
                   for i in range(TCH)]
            for j in range(NJ):
                for i in range(TCH):
                    s4_mm(psA[i], i, 0, j, j == 0, j == NJ - 1)
            stash = []
            for i in range(TCH):
                st = stashp.tile([P, HALF], f32, name=f"st{i}", tag="st")
                nc.vector.tensor_copy(out=st[:], in_=psA[i][:])
                stash.append(st)
            # h1 (j0-6), groups left open for the deferred block.
            psB = [ps.tile([P, HALF], f32, name=f"pB{i}", tag="ps")
                   for i in range(TCH)]
            for j in range(NJ):
                for i in range(TCH):
                    s4_mm(psB[i], i, 1, j, j == 0, False)
            # After the last AllReduce lands: finalize h1 (j7).
            obs = []
            for i in range(TCH):
                s4_mm(psB[i], i, 1, DB - 1, False, True)
                ob = outp.tile([P, D], f32, name=f"ob{i}", tag="ob")
                if i % 2 == 0:
                    nc.vector.tensor_copy(out=ob[:, HALF:], in_=psB[i][:])
                else:
                    nc.scalar.activation(ob[:, HALF:], psB[i][:], ACT_COPY)
                obs.append(ob)
            # h0 finalize: j7 into fresh PSUM, add the stash, one DMA per
            # token chunk.
            for i in range(TCH):
                pc = ps.tile([P, HALF], f32, name=f"pC{i}", tag="ps")
                s4_mm(pc, i, 0, DB - 1, True, True)
                ob = obs[i]
                nc.vector.tensor_tensor(
                    out=ob[:, :HALF], in0=pc[:], in1=stash[i][:],
                    op=mybir.AluOpType.add)
                nc.sync.dma_start(
                    out=out_d.ap()[i * P:(i + 1) * P, :], in_=ob[:])

    nc.compile()
    return nc


def _get_nc():
    if "nc" not in _CACHE:
        _CACHE["nc"] = _build_nc()
    return _CACHE["nc"]


def make_in_maps(inputs):
    import ml_dtypes

    bf16 = ml_dtypes.bfloat16
    x = np.asarray(inputs["x"], dtype=np.float32).reshape(B * S, D)
    keys = np.asarray(inputs["keys"], dtype=np.float32)
    vals = np.asarray(inputs["vals"], dtype=np.float32)

    col0 = [b * P for b in PH_J0] + [D]
    in_maps = []
    for c in range(NCORES):
        ksh = keys[c * KM:(c + 1) * KM].astype(bf16)
        m = {
            "vs": vals[c * KM:(c + 1) * KM].astype(bf16),
            "xT": np.ascontiguousarray(
                x[c * T:(c + 1) * T].T).astype(bf16),
        }
        for p in range(4):
            m[f"ks{p}"] = np.ascontiguousarray(ksh[:, col0[p]:col0[p + 1]])
        in_maps.append(m)
    return in_maps


def kernel(**inputs):
    from concourse.bass_utils import run_bass_kernel_spmd

    nc = _get_nc()
    in_maps = make_in_maps(inputs)
    res = run_bass_kernel_spmd(nc, in_maps, list(range(NCORES)))
    out = np.concatenate([res.results[c]["out"] for c in range(NCORES)],
                         axis=0)
    return out.reshape(B, S, D).astype(np.float32)
